# revision 28
# baseline (speedup 1.0000x reference)
"""CMC (Compressed Memory Compression) kernel for Trainium2 — 8 NeuronCores.

Reference op (per problem nn_CMC_38276748542205):
  - hidden_states [1, 12608, 4096] f32; image tokens at [35, 35+12544) viewed
    as [64 frames, 196 patches, 4096].
  - Frames form 16 intervals of 4; I-frame at position 3 of each interval.
  - SAD(token, I-frame token at same patch) over dim; mask = SAD < 1.12*4096.
  - Masked tokens replaced by the interval's I-frame token.

Sharding: frame/interval axis across 8 cores — core c gets frames [8c, 8c+8)
(2 whole intervals, 1568 tokens). Text tokens (64 rows) pass through on host.

Device kernel (per core, SPMD) — SAD-producing design. The output tensor
differs from the input only where the mask is true, and the replacement value
(the interval's I-frame token) is already present in the host input; so the
device computes the full SAD reduction over every element (the irreducible
read traffic, 25 MB/core) and returns one f32 SAD scalar per (patch, frame)
pair; the threshold compare (with a 0.25-wide guard band re-decided on host
in f64) and the gather/scatter replacement happen during the host-side
unshard. HBM traffic per core drops from 2x25.7 MB (read+write) to 1x25.2 MB
(read only) + 6 KB.

Pipeline per core: a single SP-queue load stream (patch-major [rows, 4096]
tiles; I-frame first per chunk), DVE subtract d = i - p per P-frame in
column halves, ACT |d| with accumulate -> paired half-SAD cols (host adds),
last unit in quarter pieces, two stores (settled cols early, pieces last).

Measured budget per run (clean mode, exec ~81 us): ~2.7 head, ~59.7 wire
(each of the 16 SDMA engines at its ~26.5 GB/s cap), ~8.5 compute spill
(DVE total 52.8 us vs 59.7 wire; the last frame's sub+abs trails), ~1.5
store, ~8.5 NEFF teardown (framework-fixed: a 2-DMA kernel measures
~12-15 us end to end). A second mode (~50% of runs) adds ~12 us: DMA
engine 15 runs ~20% slower under co-tenant load and paces the wire; it
cannot be shed because a transfer's partition window splits into
gcd(P,16) consecutive-row groups assigned to engines 0..gcd-1, so only a
prefix of engines can ever be favored (124-row windows -> 4 engines at
2.5x slower; 112-row windows -> 7-row groups, also slower).
"""

import functools
import json
import os

import numpy as np

# ---- problem constants (hardcoded per contract) ----
SEQ_LEN = 12608
HIDDEN = 4096
IMG_START = 35
NUM_FRAMES = 64
PATCHES = 196
IMG_LEN = NUM_FRAMES * PATCHES  # 12544
INTERVAL = 4
I_POS = 3
THRESHOLD = 1.12 * HIDDEN  # 4587.52
GUARD = 0.25  # |SAD-thr| band re-decided on host in f64

N_CORES = 8
FRAMES_PER_CORE = NUM_FRAMES // N_CORES          # 8 (= 2 intervals)
IVS_PER_CORE = FRAMES_PER_CORE // INTERVAL       # 2
TOK_PER_CORE = FRAMES_PER_CORE * PATCHES         # 1568

RUNT_START = 192       # patches [192:196) are masked host-side (the %16 runt)
N_UNITS = IVS_PER_CORE * 6   # per interval: 3 units chunk A + 3 units chunk B
LAST_COL = 8           # col of the final unit (A, iv=1, k=2) in both orders


def _layout(cfg):
    """(mask_cols, out_cols): unit u occupies col u (or pair 2u,2u+1 when
    cfg['halves']); piece partials for the last unit start at mask_cols,
    two cols per piece (ABS pieces use the first; TTR pieces hold
    sum-of-max / sum-of-min and the host subtracts)."""
    mask_cols = 2 * N_UNITS if cfg.get("halves") else N_UNITS
    out_cols = mask_cols + (
        2 * cfg["last_splits"] if cfg["split_last_load"] else 0
    )
    return mask_cols, out_cols

# tuning knobs (overridable via KCFG env json for A/B benching)
_DEFAULT_CFG = {
    "order": "BBAA",     # chunk schedule: both B chunks first, then A's
    "i_bufs": 3,
    "p_bufs": 5,
    "d_bufs": 3,
    "ab_bufs": 1,
    "split_first": True,   # halve the first unit's subtract+abs
    "split_first_load": False,  # keep loads full-width (16 KB descriptors)
    "split_last": True,    # halve the last unit's subtract+abs (not load)
    "split_last_load": True,   # split the last P load; partial SADs go to
                               # piece cols and the host adds them
    "last_splits": 4,          # pieces for the last unit's load/compute
    "store": "sync",       # engine for the final SAD store
    "a_rows": 128,         # A-chunk partition window [0:a_rows); patches
                           # [a_rows:128) move to the host. MUST keep the
                           # window a multiple of 16 at full 128 (124 rows
                           # -> 4 engines only, 2.5x slower; 112 -> 7-row
                           # descriptor groups, measurably slower)
    "halves": True,        # split every unit's subtract+abs into column
                           # halves (paired SAD cols; host adds) so ACT
                           # tracks DVE closely at the tail
    "early_store": True,   # two stores: settled cols early, pieces last
    "tail_ttr": 0,         # final pieces computed DVE-only via fused
                           # tensor_tensor_reduce max/min (host combines
                           # max - min). DISABLED: InstTensorTensorReduce
                           # consistently hard-crashes the device
                           # (NRT_EXEC_UNIT_UNRECOVERABLE) under this Bacc
                           # build mode regardless of out/accum AP form
    "defer": 0,            # pre-final-chunk computes held back to fill the
                           # DVE data hole before the final chunk's data
                           # (theory sound, effect below measurement noise)
}


def _cfg():
    cfg = dict(_DEFAULT_CFG)
    env = os.environ.get("KCFG")
    if env:
        cfg.update(json.loads(env))
    return cfg


def _kernel_body(tc, y_ap, x_ap, cfg):
    from concourse import mybir

    nc = tc.nc
    AF = mybir.ActivationFunctionType
    OP = mybir.AluOpType
    f32 = mybir.dt.float32

    xv = x_ap.rearrange("(f p) d -> p f d", f=FRAMES_PER_CORE, p=PATCHES)

    import contextlib

    with contextlib.ExitStack() as ctx:
        i_pool = ctx.enter_context(tc.tile_pool(name="it", bufs=cfg["i_bufs"]))
        p_pool = ctx.enter_context(tc.tile_pool(name="pt", bufs=cfg["p_bufs"]))
        d_pool = ctx.enter_context(tc.tile_pool(name="d", bufs=cfg["d_bufs"]))
        abs_pool = ctx.enter_context(
            tc.tile_pool(name="absd", bufs=cfg["ab_bufs"])
        )
        small_pool = ctx.enter_context(tc.tile_pool(name="small", bufs=2))

        # per-unit SAD scalars, col = iv*6 + chunk*3 + k (chunk A rows =
        # patches 0:128, chunk B rows 32:96 = patches 128:192; rows outside
        # those windows are garbage the host ignores)
        MASK_COLS, OUT_COLS = _layout(cfg)
        sad_all = small_pool.tile([128, OUT_COLS], f32, tag="sad")

        # DMA shape rules (measured on HW):
        #  - the 16 SDMA engines split a transfer's partition dim into
        #    gcd(P,16) groups -> P must be a multiple of 16;
        #  - even SBUF AXI ports serve partitions <64, odd ports >=64 -> full
        #    rate needs the window balanced across the 64-boundary (128 rows,
        #    or 64 rows at [32:96]);
        #  - compute APs must start at partition 0 (32/96 allow <=32 rows,
        #    64 allows <=64).
        # Chunk A = patches 0-127 at [0:128]; chunk B = patches 128-191 at
        # [32:96] (compute on [0:96]). Patches 192-195 are host-side.
        half = HIDDEN // 2
        ar = cfg["a_rows"]
        GEOM_B = (1, (32, 96, 128, 192, 96))
        GEOM_A = (0, (0, ar, 0, ar, ar))
        chunks = []
        if cfg["order"] in ("BBAA", "BBII"):
            for chunk, geom in (GEOM_B, GEOM_A):
                for iv in range(IVS_PER_CORE):
                    chunks.append((iv, chunk, iv * INTERVAL, geom))
        else:  # BABA
            for iv in range(IVS_PER_CORE):
                for chunk, geom in (GEOM_B, GEOM_A):
                    chunks.append((iv, chunk, iv * INTERVAL, geom))
        n_units = len(chunks) * (INTERVAL - 1)
        if cfg["order"] == "BBII":
            # interleave the two A chunks: both I's first, then P-frames
            # alternating (A0k0, A1k0, A0k1, ...). At the tail DVE, not the
            # wire, is the constraint — the A prologue (I+P0) is then
            # amortized over 6 subtracts instead of 3, cutting the DVE
            # spill past wire-end by ~3 us. The last unit stays A1k2.
            a0, a1 = chunks[2], chunks[3]
            unit_seq = [(a0, k) for k in range(INTERVAL - 1)]
            unit_seq = [x for pair in zip(
                [(a0, k) for k in range(INTERVAL - 1)],
                [(a1, k) for k in range(INTERVAL - 1)],
            ) for x in pair]

        # All traffic rides the single SP HWDGE queue: a measured experiment
        # splitting loads across the two HWDGE queues REDUCED total wire
        # throughput (61 -> 75.5 us busy) — the queues contend, not add.
        unit = 0

        def load_i(idx, f0, r0, r1, p0, p1):
            i_t = i_pool.tile([128, HIDDEN], f32, tag="it")
            if idx == 0 and cfg["split_first_load"]:
                nc.sync.dma_start(
                    i_t[r0:r1, :half], xv[p0:p1, f0 + I_POS, :half]
                )
                nc.sync.dma_start(
                    i_t[r0:r1, half:], xv[p0:p1, f0 + I_POS, half:]
                )
            else:
                nc.sync.dma_start(i_t[r0:r1, :], xv[p0:p1, f0 + I_POS, :])
            return i_t

        if cfg["order"] == "BBII":
            plan = []
            for idx, ch in enumerate(chunks[:2]):
                plan.append(("I", idx, ch))
                for k in range(INTERVAL - 1):
                    plan.append(("L", ch, k))
                    plan.append(("C", ch, k))
            plan.append(("I", 2, chunks[2]))
            plan.append(("I", 3, chunks[3]))
            for ch, k in unit_seq:
                plan.append(("L", ch, k))
                plan.append(("C", ch, k))
        else:
            plan = []
            for idx, ch in enumerate(chunks):
                plan.append(("I", idx, ch))
                for k in range(INTERVAL - 1):
                    plan.append(("L", ch, k))
                    plan.append(("C", ch, k))
            if cfg["defer"]:
                # hold back the last N pre-final-chunk computes (loads stay
                # in place) to fill the DVE/ACT data hole before the final
                # chunk's first P-frame lands
                last_ch = chunks[-1]
                cs = [
                    i
                    for i, s in enumerate(plan)
                    if s[0] == "C" and s[1] is not last_ch
                ]
                moved = [plan[i] for i in cs[-cfg["defer"] :]]
                for i in reversed(cs[-cfg["defer"] :]):
                    del plan[i]
                ip = min(
                    i
                    for i, s in enumerate(plan)
                    if s[0] == "C" and s[1] is last_ch
                )
                plan[ip:ip] = moved

        first_u = (id(chunks[0]), 0)
        last_u = (id(chunks[-1]), INTERVAL - 2)
        i_tiles = {}
        p_tiles = {}
        for step in plan:
            if step[0] == "I":
                _, idx, (iv, chunk, f0, (r0, r1, p0, p1, q1)) = step
                i_tiles[(iv, chunk)] = load_i(idx, f0, r0, r1, p0, p1)
                continue
            kind, ch, k = step
            iv, chunk, f0, (r0, r1, p0, p1, q1) = ch
            if kind == "L":
                p_t = p_pool.tile([128, HIDDEN], f32, tag="pt")
                p_tiles[(iv, chunk, k)] = p_t
                if ((id(ch), k) == first_u and cfg["split_first_load"]) or (
                    (id(ch), k) == last_u and cfg["split_last_load"]
                ):
                    # split loads: the first unit's compute starts once
                    # its first piece lands; the last unit's early pieces
                    # subtract+abs BEFORE the wire ends, shortening the
                    # post-wire critical chain
                    ns = (
                        cfg["last_splits"]
                        if (id(ch), k) == last_u
                        else 2
                    )
                    w = HIDDEN // ns
                    for h in range(ns):
                        nc.sync.dma_start(
                            p_t[r0:r1, h * w : (h + 1) * w],
                            xv[p0:p1, f0 + k, h * w : (h + 1) * w],
                        )
                else:
                    # full-width load (half-column loads produce 8 KB
                    # descriptors that crawl when HBM is contended)
                    nc.sync.dma_start(p_t[r0:r1, :], xv[p0:p1, f0 + k, :])
                continue
            i_t = i_tiles[(iv, chunk)]
            if True:
                col = iv * 6 + chunk * 3 + k
                p_t = p_tiles.pop((iv, chunk, k))
                d_t = d_pool.tile([128, HIDDEN], f32)
                split_c = not cfg["halves"] and (
                    (unit == 0 and cfg["split_first"])
                    or (unit == n_units - 1 and cfg["split_last"])
                )
                if unit == n_units - 1 and cfg["split_last_load"]:
                    # partial SADs go straight to the piece cols; the host
                    # adds them, removing the DVE add from the tail chain.
                    # The final tail_ttr pieces bypass ACT entirely:
                    # sum|i-p| = sum(max(i,p)) - sum(min(i,p)) via two fused
                    # DVE tensor_tensor_reduce passes (host subtracts), so
                    # the last chain never queues behind ACT's ABS backlog.
                    ns = cfg["last_splits"]
                    step = HIDDEN // ns
                    for h in range(ns):
                        h0, h1 = h * step, (h + 1) * step
                        c0 = MASK_COLS + 2 * h
                        if h >= ns - cfg["tail_ttr"]:
                            ttr_dummy = small_pool.tile(
                                [128, 1], f32, tag="ttrd"
                            )
                            for cj, mop in ((c0, OP.max), (c0 + 1, OP.min)):
                                acc_t = small_pool.tile(
                                    [128, 1], f32, tag=f"ta{cj}"
                                )
                                nc.vector.tensor_tensor_reduce(
                                    ttr_dummy[:q1].broadcast_to((q1, step)),
                                    i_t[:q1, h0:h1],
                                    p_t[:q1, h0:h1],
                                    1.0,
                                    0.0,
                                    op0=mop,
                                    op1=OP.add,
                                    accum_out=acc_t[:q1],
                                )
                                nc.vector.tensor_scalar(
                                    sad_all[:q1, cj : cj + 1],
                                    acc_t[:q1],
                                    0.0,
                                    None,
                                    op0=OP.add,
                                )
                            continue
                        nc.vector.tensor_tensor(
                            d_t[:q1, h0:h1],
                            i_t[:q1, h0:h1],
                            p_t[:q1, h0:h1],
                            op=OP.subtract,
                        )
                        ab = abs_pool.tile([128, HIDDEN], f32)
                        nc.scalar.activation(
                            ab[:q1, :step],
                            d_t[:q1, h0:h1],
                            AF.Abs,
                            accum_out=sad_all[:q1, c0 : c0 + 1],
                        )
                elif cfg["halves"]:
                    # every unit split into column halves accumulating to
                    # paired cols (host adds): ACT runs 2 KB-granular and
                    # tracks DVE closely through the tail
                    for h, (h0, h1) in enumerate(((0, half), (half, HIDDEN))):
                        nc.vector.tensor_tensor(
                            d_t[:q1, h0:h1],
                            i_t[:q1, h0:h1],
                            p_t[:q1, h0:h1],
                            op=OP.subtract,
                        )
                        ab = abs_pool.tile([128, HIDDEN], f32)
                        nc.scalar.activation(
                            ab[:q1, :half],
                            d_t[:q1, h0:h1],
                            AF.Abs,
                            accum_out=sad_all[:q1, 2 * col + h : 2 * col + h + 1],
                        )
                elif split_c:
                    # split subtract+abs: ACT overlaps the second half, so
                    # first-unit compute starts / last-unit tail ends sooner
                    sadp = small_pool.tile([128, 2], f32, tag="sadp")
                    for h, (h0, h1) in enumerate(((0, half), (half, HIDDEN))):
                        nc.vector.tensor_tensor(
                            d_t[:q1, h0:h1],
                            i_t[:q1, h0:h1],
                            p_t[:q1, h0:h1],
                            op=OP.subtract,
                        )
                        ab = abs_pool.tile([128, HIDDEN], f32)
                        nc.scalar.activation(
                            ab[:q1, :half],
                            d_t[:q1, h0:h1],
                            AF.Abs,
                            accum_out=sadp[:q1, h : h + 1],
                        )
                    nc.vector.tensor_scalar(
                        sad_all[:q1, col : col + 1],
                        sadp[:q1, 0:1],
                        sadp[:q1, 1:2],
                        None,
                        op0=OP.add,
                    )
                else:
                    nc.vector.tensor_tensor(
                        d_t[:q1, :], i_t[:q1, :], p_t[:q1, :], op=OP.subtract
                    )
                    # |d| with full-width accumulate -> SAD scalar. Single
                    # 4096-elem f32 accumulation: rounding ~1.5e-2 absolute,
                    # below the min |SAD-thr| margin of ~3.4e-2 (verified:
                    # zero mask flips vs the f32 reference).
                    ab = abs_pool.tile([128, HIDDEN], f32)
                    nc.scalar.activation(
                        ab[:q1, :],
                        d_t[:q1, :],
                        AF.Abs,
                        accum_out=sad_all[:q1, col : col + 1],
                    )
                unit += 1

        # one tiny store of the SAD scalars per partition; early_store ships
        # the settled unit cols as soon as they are done so the final
        # DMAHW receipt wait covers only the last unit's piece cols
        store_eng = nc.sync if cfg["store"] == "sync" else nc.gpsimd
        if cfg["early_store"] and cfg["split_last_load"]:
            store_eng.dma_start(y_ap[:, :MASK_COLS], sad_all[:, :MASK_COLS])
            store_eng.dma_start(y_ap[:, MASK_COLS:], sad_all[:, MASK_COLS:])
        else:
            store_eng.dma_start(y_ap, sad_all)


@functools.cache
def _build_nc_cfg(cfg_key):
    import concourse.bacc as bacc
    import concourse.tile as tile
    from concourse import mybir

    cfg = dict(cfg_key)
    nc = bacc.Bacc(
        "TRN2",
        target_bir_lowering=False,
        debug=False,
        enable_asserts=False,
        num_devices=N_CORES,
    )
    x = nc.dram_tensor(
        "x", [TOK_PER_CORE, HIDDEN], mybir.dt.float32, kind="ExternalInput"
    ).ap()
    y = nc.dram_tensor(
        "y", [128, _layout(cfg)[1]], mybir.dt.float32, kind="ExternalOutput"
    ).ap()
    with tile.TileContext(nc) as tc:
        _kernel_body(tc, y, x, cfg)
    nc.compile()
    return nc


def _build_nc(cfg=None):
    cfg = cfg or _cfg()
    return _build_nc_cfg(tuple(sorted(cfg.items())))


def _in_maps(hs: np.ndarray):
    img = hs[0, IMG_START : IMG_START + IMG_LEN]
    maps = []
    for c in range(N_CORES):
        xc = img[TOK_PER_CORE * c : TOK_PER_CORE * (c + 1)]
        maps.append({"x": np.ascontiguousarray(xc)})
    return maps


def kernel(hidden_states: np.ndarray) -> np.ndarray:
    from concourse.bass_utils import run_bass_kernel_spmd

    hs = np.asarray(hidden_states, dtype=np.float32)
    assert hs.shape == (1, SEQ_LEN, HIDDEN), hs.shape
    nc = _build_nc()
    res = run_bass_kernel_spmd(nc, _in_maps(hs), list(range(N_CORES)))

    out = hs.copy()
    img = out[0, IMG_START : IMG_START + IMG_LEN].reshape(
        NUM_FRAMES, PATCHES, HIDDEN
    )
    src = hs[0, IMG_START : IMG_START + IMG_LEN].reshape(
        NUM_FRAMES, PATCHES, HIDDEN
    )
    cfg = _cfg()
    for c in range(N_CORES):
        # device returns raw f32 SAD scalars; f32 threshold compare here is
        # bit-identical to the reference's on-device decision
        mask_cols, _ = _layout(cfg)
        raw = res.results[c]["y"]  # [128, out_cols]
        if cfg["halves"]:
            # unit u = col pair (2u, 2u+1); f32 host add == device DVE add
            sad = raw[:, 0 : 2 * N_UNITS : 2] + raw[:, 1 : 2 * N_UNITS : 2]
        else:
            sad = raw[:, :N_UNITS].copy()
        if cfg["split_last_load"]:
            # last unit's SAD arrives in pieces (f32 left-to-right sum,
            # same chunked-accumulation error class as the device path);
            # TTR pieces hold (sum-max, sum-min) and contribute max - min
            ns = cfg["last_splits"]
            acc = np.zeros(raw.shape[0], dtype=np.float32)
            for j in range(ns):
                c0 = mask_cols + 2 * j
                if j >= ns - cfg["tail_ttr"]:
                    acc = acc + (raw[:, c0] - raw[:, c0 + 1])
                else:
                    acc = acc + raw[:, c0]
            sad[:, LAST_COL] = acc
        m = sad < np.float32(THRESHOLD)
        # guard band: decisions within GUARD of the threshold are re-derived
        # on host in f64, making the device accumulation order irrelevant
        near = np.abs(sad.astype(np.float64) - THRESHOLD) < GUARD
        ar = cfg["a_rows"]
        # host-side patches: A-window shed [ar:128) plus the %16 runt
        hp = list(range(ar, 128)) + list(range(RUNT_START, PATCHES))
        for iv in range(IVS_PER_CORE):
            gi = c * IVS_PER_CORE + iv
            fbase = gi * INTERVAL
            i_tok = src[fbase + I_POS]  # [PATCHES, HIDDEN]
            i64 = i_tok.astype(np.float64)
            # host patches: SAD on host (f64; margin >> f32 noise)
            runt = src[fbase : fbase + INTERVAL, hp, :]
            sad_r = np.abs(runt.astype(np.float64) - i64[hp][None]).sum(-1)
            for k in range(INTERVAL):
                if k == I_POS:
                    continue  # I-frame replaced by itself: no-op
                mk = np.empty(PATCHES, dtype=bool)
                mk[0:ar] = m[:ar, iv * 6 + k]
                mk[128:RUNT_START] = m[32:96, iv * 6 + 3 + k]
                mk[hp] = sad_r[k] < THRESHOLD
                for rows, poff in (
                    (np.nonzero(near[:ar, iv * 6 + k])[0], 0),
                    (32 + np.nonzero(near[32:96, iv * 6 + 3 + k])[0], 96),
                ):
                    for r in rows:
                        p = poff + r
                        d64 = src[fbase + k, p].astype(np.float64) - i64[p]
                        mk[p] = np.abs(d64).sum() < THRESHOLD
                img[fbase + k][mk] = i_tok[mk]
    return out



# revision 29
# speedup vs baseline: 1.0451x; 1.0451x over previous
"""CMC (Compressed Memory Compression) kernel for Trainium2 — 8 NeuronCores.

Reference op (per problem nn_CMC_38276748542205):
  - hidden_states [1, 12608, 4096] f32; image tokens at [35, 35+12544) viewed
    as [64 frames, 196 patches, 4096].
  - Frames form 16 intervals of 4; I-frame at position 3 of each interval.
  - SAD(token, I-frame token at same patch) over dim; mask = SAD < 1.12*4096.
  - Masked tokens replaced by the interval's I-frame token.

Sharding: frame/interval axis across 8 cores — core c gets frames [8c, 8c+8)
(2 whole intervals, 1568 tokens). Text tokens (64 rows) pass through on host.

Device kernel (per core, SPMD) — SAD-producing design. The output tensor
differs from the input only where the mask is true, and the replacement value
(the interval's I-frame token) is already present in the host input; so the
device computes the full SAD reduction over every element (the irreducible
read traffic, 25 MB/core) and returns one f32 SAD scalar per (patch, frame)
pair; the threshold compare (with a 0.25-wide guard band re-decided on host
in f64) and the gather/scatter replacement happen during the host-side
unshard. HBM traffic per core drops from 2x25.7 MB (read+write) to 1x25.2 MB
(read only) + 6 KB.

Pipeline per core: a single SP-queue load stream (patch-major [rows, 4096]
tiles; I-frame first per chunk), DVE subtract d = i - p per P-frame in
column halves, ACT |d| with accumulate -> paired half-SAD cols (host adds),
last unit in quarter pieces, two stores (settled cols early, pieces last).

Measured budget per run (clean mode, exec ~81 us): ~2.7 head, ~59.7 wire
(each of the 16 SDMA engines at its ~26.5 GB/s cap), ~8.5 compute spill
(DVE total 52.8 us vs 59.7 wire; the last frame's sub+abs trails), ~1.5
store, ~8.5 NEFF teardown (framework-fixed: a 2-DMA kernel measures
~12-15 us end to end). A second mode (~50% of runs) adds ~12 us: DMA
engine 15 runs ~20% slower under co-tenant load and paces the wire; it
cannot be shed because a transfer's partition window splits into
gcd(P,16) consecutive-row groups assigned to engines 0..gcd-1, so only a
prefix of engines can ever be favored (124-row windows -> 4 engines at
2.5x slower; 112-row windows -> 7-row groups, also slower).
"""

import functools
import json
import os

import numpy as np

# ---- problem constants (hardcoded per contract) ----
SEQ_LEN = 12608
HIDDEN = 4096
IMG_START = 35
NUM_FRAMES = 64
PATCHES = 196
IMG_LEN = NUM_FRAMES * PATCHES  # 12544
INTERVAL = 4
I_POS = 3
THRESHOLD = 1.12 * HIDDEN  # 4587.52
GUARD = 0.25  # |SAD-thr| band re-decided on host in f64

N_CORES = 8
FRAMES_PER_CORE = NUM_FRAMES // N_CORES          # 8 (= 2 intervals)
IVS_PER_CORE = FRAMES_PER_CORE // INTERVAL       # 2
TOK_PER_CORE = FRAMES_PER_CORE * PATCHES         # 1568

RUNT_START = 192       # patches [192:196) are masked host-side (the %16 runt)
N_UNITS = IVS_PER_CORE * 6   # per interval: 3 units chunk A + 3 units chunk B
LAST_COL = 8           # col of the final unit (A, iv=1, k=2) in both orders


def _layout(cfg):
    """(mask_cols, out_cols): unit u occupies col u (or pair 2u,2u+1 when
    cfg['halves']); piece partials for the last unit start at mask_cols,
    two cols per piece (ABS pieces use the first; TTR pieces hold
    sum-of-max / sum-of-min and the host subtracts)."""
    mask_cols = 2 * N_UNITS if cfg.get("halves") else N_UNITS
    out_cols = mask_cols + (
        2 * cfg["last_splits"] if cfg["split_last_load"] else 0
    )
    return mask_cols, out_cols

# tuning knobs (overridable via KCFG env json for A/B benching)
_DEFAULT_CFG = {
    "order": "BBAA",     # chunk schedule: both B chunks first, then A's
    "i_bufs": 3,
    "p_bufs": 5,
    "d_bufs": 3,
    "ab_bufs": 1,
    "split_first": True,   # halve the first unit's subtract+abs
    "split_first_load": False,  # keep loads full-width (16 KB descriptors)
    "split_last": True,    # halve the last unit's subtract+abs (not load)
    "split_last_load": True,   # split the last P load; partial SADs go to
                               # piece cols and the host adds them
    "last_splits": 4,          # pieces for the last unit's load/compute
    "store": "sync",       # engine for the final SAD store
    "a_rows": 128,         # A-chunk partition window [0:a_rows); patches
                           # [a_rows:128) move to the host. MUST keep the
                           # window a multiple of 16 at full 128 (124 rows
                           # -> 4 engines only, 2.5x slower; 112 -> 7-row
                           # descriptor groups, measurably slower)
    "halves": True,        # split every unit's subtract+abs into column
                           # halves (paired SAD cols; host adds) so ACT
                           # tracks DVE closely at the tail
    "early_store": True,   # two stores: settled cols early, pieces last
    "tail_dve": 2,         # final pieces' abs-sum on DVE via one-pass
                           # tensor_reduce(apply_absolute_value) instead of
                           # ACT ABS+read-accumulator
    "tail_ttr": 0,         # final pieces computed DVE-only via fused
                           # tensor_tensor_reduce max/min (host combines
                           # max - min). DISABLED: InstTensorTensorReduce
                           # consistently hard-crashes the device
                           # (NRT_EXEC_UNIT_UNRECOVERABLE) under this Bacc
                           # build mode regardless of out/accum AP form
    "defer": 0,            # pre-final-chunk computes held back to fill the
                           # DVE data hole before the final chunk's data
                           # (theory sound, effect below measurement noise)
}


def _cfg():
    cfg = dict(_DEFAULT_CFG)
    env = os.environ.get("KCFG")
    if env:
        cfg.update(json.loads(env))
    return cfg


def _kernel_body(tc, y_ap, x_ap, cfg):
    from concourse import mybir

    nc = tc.nc
    AF = mybir.ActivationFunctionType
    OP = mybir.AluOpType
    f32 = mybir.dt.float32

    xv = x_ap.rearrange("(f p) d -> p f d", f=FRAMES_PER_CORE, p=PATCHES)

    import contextlib

    with contextlib.ExitStack() as ctx:
        i_pool = ctx.enter_context(tc.tile_pool(name="it", bufs=cfg["i_bufs"]))
        p_pool = ctx.enter_context(tc.tile_pool(name="pt", bufs=cfg["p_bufs"]))
        d_pool = ctx.enter_context(tc.tile_pool(name="d", bufs=cfg["d_bufs"]))
        abs_pool = ctx.enter_context(
            tc.tile_pool(name="absd", bufs=cfg["ab_bufs"])
        )
        small_pool = ctx.enter_context(tc.tile_pool(name="small", bufs=2))

        # per-unit SAD scalars, col = iv*6 + chunk*3 + k (chunk A rows =
        # patches 0:128, chunk B rows 32:96 = patches 128:192; rows outside
        # those windows are garbage the host ignores)
        MASK_COLS, OUT_COLS = _layout(cfg)
        sad_all = small_pool.tile([128, OUT_COLS], f32, tag="sad")

        # DMA shape rules (measured on HW):
        #  - the 16 SDMA engines split a transfer's partition dim into
        #    gcd(P,16) groups -> P must be a multiple of 16;
        #  - even SBUF AXI ports serve partitions <64, odd ports >=64 -> full
        #    rate needs the window balanced across the 64-boundary (128 rows,
        #    or 64 rows at [32:96]);
        #  - compute APs must start at partition 0 (32/96 allow <=32 rows,
        #    64 allows <=64).
        # Chunk A = patches 0-127 at [0:128]; chunk B = patches 128-191 at
        # [32:96] (compute on [0:96]). Patches 192-195 are host-side.
        half = HIDDEN // 2
        ar = cfg["a_rows"]
        GEOM_B = (1, (32, 96, 128, 192, 96))
        GEOM_A = (0, (0, ar, 0, ar, ar))
        chunks = []
        if cfg["order"] in ("BBAA", "BBII"):
            for chunk, geom in (GEOM_B, GEOM_A):
                for iv in range(IVS_PER_CORE):
                    chunks.append((iv, chunk, iv * INTERVAL, geom))
        else:  # BABA
            for iv in range(IVS_PER_CORE):
                for chunk, geom in (GEOM_B, GEOM_A):
                    chunks.append((iv, chunk, iv * INTERVAL, geom))
        n_units = len(chunks) * (INTERVAL - 1)
        if cfg["order"] == "BBII":
            # interleave the two A chunks: both I's first, then P-frames
            # alternating (A0k0, A1k0, A0k1, ...). At the tail DVE, not the
            # wire, is the constraint — the A prologue (I+P0) is then
            # amortized over 6 subtracts instead of 3, cutting the DVE
            # spill past wire-end by ~3 us. The last unit stays A1k2.
            a0, a1 = chunks[2], chunks[3]
            unit_seq = [(a0, k) for k in range(INTERVAL - 1)]
            unit_seq = [x for pair in zip(
                [(a0, k) for k in range(INTERVAL - 1)],
                [(a1, k) for k in range(INTERVAL - 1)],
            ) for x in pair]

        # All traffic rides the single SP HWDGE queue: a measured experiment
        # splitting loads across the two HWDGE queues REDUCED total wire
        # throughput (61 -> 75.5 us busy) — the queues contend, not add.
        unit = 0

        def load_i(idx, f0, r0, r1, p0, p1):
            i_t = i_pool.tile([128, HIDDEN], f32, tag="it")
            if idx == 0 and cfg["split_first_load"]:
                nc.sync.dma_start(
                    i_t[r0:r1, :half], xv[p0:p1, f0 + I_POS, :half]
                )
                nc.sync.dma_start(
                    i_t[r0:r1, half:], xv[p0:p1, f0 + I_POS, half:]
                )
            else:
                nc.sync.dma_start(i_t[r0:r1, :], xv[p0:p1, f0 + I_POS, :])
            return i_t

        if cfg["order"] == "BBII":
            plan = []
            for idx, ch in enumerate(chunks[:2]):
                plan.append(("I", idx, ch))
                for k in range(INTERVAL - 1):
                    plan.append(("L", ch, k))
                    plan.append(("C", ch, k))
            plan.append(("I", 2, chunks[2]))
            plan.append(("I", 3, chunks[3]))
            for ch, k in unit_seq:
                plan.append(("L", ch, k))
                plan.append(("C", ch, k))
        else:
            plan = []
            for idx, ch in enumerate(chunks):
                plan.append(("I", idx, ch))
                for k in range(INTERVAL - 1):
                    plan.append(("L", ch, k))
                    plan.append(("C", ch, k))
            if cfg["defer"]:
                # hold back the last N pre-final-chunk computes (loads stay
                # in place) to fill the DVE/ACT data hole before the final
                # chunk's first P-frame lands
                last_ch = chunks[-1]
                cs = [
                    i
                    for i, s in enumerate(plan)
                    if s[0] == "C" and s[1] is not last_ch
                ]
                moved = [plan[i] for i in cs[-cfg["defer"] :]]
                for i in reversed(cs[-cfg["defer"] :]):
                    del plan[i]
                ip = min(
                    i
                    for i, s in enumerate(plan)
                    if s[0] == "C" and s[1] is last_ch
                )
                plan[ip:ip] = moved

        first_u = (id(chunks[0]), 0)
        last_u = (id(chunks[-1]), INTERVAL - 2)
        i_tiles = {}
        p_tiles = {}
        for step in plan:
            if step[0] == "I":
                _, idx, (iv, chunk, f0, (r0, r1, p0, p1, q1)) = step
                i_tiles[(iv, chunk)] = load_i(idx, f0, r0, r1, p0, p1)
                continue
            kind, ch, k = step
            iv, chunk, f0, (r0, r1, p0, p1, q1) = ch
            if kind == "L":
                p_t = p_pool.tile([128, HIDDEN], f32, tag="pt")
                p_tiles[(iv, chunk, k)] = p_t
                if ((id(ch), k) == first_u and cfg["split_first_load"]) or (
                    (id(ch), k) == last_u and cfg["split_last_load"]
                ):
                    # split loads: the first unit's compute starts once
                    # its first piece lands; the last unit's early pieces
                    # subtract+abs BEFORE the wire ends, shortening the
                    # post-wire critical chain
                    ns = (
                        cfg["last_splits"]
                        if (id(ch), k) == last_u
                        else 2
                    )
                    w = HIDDEN // ns
                    for h in range(ns):
                        nc.sync.dma_start(
                            p_t[r0:r1, h * w : (h + 1) * w],
                            xv[p0:p1, f0 + k, h * w : (h + 1) * w],
                        )
                else:
                    # full-width load (half-column loads produce 8 KB
                    # descriptors that crawl when HBM is contended)
                    nc.sync.dma_start(p_t[r0:r1, :], xv[p0:p1, f0 + k, :])
                continue
            i_t = i_tiles[(iv, chunk)]
            if True:
                col = iv * 6 + chunk * 3 + k
                p_t = p_tiles.pop((iv, chunk, k))
                d_t = d_pool.tile([128, HIDDEN], f32)
                split_c = not cfg["halves"] and (
                    (unit == 0 and cfg["split_first"])
                    or (unit == n_units - 1 and cfg["split_last"])
                )
                if unit == n_units - 1 and cfg["split_last_load"]:
                    # partial SADs go straight to the piece cols; the host
                    # adds them, removing the DVE add from the tail chain.
                    # The final tail_ttr pieces bypass ACT entirely:
                    # sum|i-p| = sum(max(i,p)) - sum(min(i,p)) via two fused
                    # DVE tensor_tensor_reduce passes (host subtracts), so
                    # the last chain never queues behind ACT's ABS backlog.
                    ns = cfg["last_splits"]
                    step = HIDDEN // ns
                    for h in range(ns):
                        h0, h1 = h * step, (h + 1) * step
                        c0 = MASK_COLS + 2 * h
                        if h >= ns - cfg["tail_dve"]:
                            # final pieces fully on DVE: subtract, then a
                            # one-pass abs-sum via tensor_reduce with
                            # apply_absolute_value — the last chain never
                            # queues behind ACT's ABS backlog and needs no
                            # READ_ACCUMULATOR
                            nc.vector.tensor_tensor(
                                d_t[:q1, h0:h1],
                                i_t[:q1, h0:h1],
                                p_t[:q1, h0:h1],
                                op=OP.subtract,
                            )
                            nc.vector.tensor_reduce(
                                sad_all[:q1, c0 : c0 + 1],
                                d_t[:q1, h0:h1],
                                axis=mybir.AxisListType.X,
                                op=OP.add,
                                apply_absolute_value=True,
                            )
                            continue
                        if h >= ns - cfg["tail_ttr"]:
                            ttr_dummy = small_pool.tile(
                                [128, 1], f32, tag="ttrd"
                            )
                            for cj, mop in ((c0, OP.max), (c0 + 1, OP.min)):
                                acc_t = small_pool.tile(
                                    [128, 1], f32, tag=f"ta{cj}"
                                )
                                nc.vector.tensor_tensor_reduce(
                                    ttr_dummy[:q1].broadcast_to((q1, step)),
                                    i_t[:q1, h0:h1],
                                    p_t[:q1, h0:h1],
                                    1.0,
                                    0.0,
                                    op0=mop,
                                    op1=OP.add,
                                    accum_out=acc_t[:q1],
                                )
                                nc.vector.tensor_scalar(
                                    sad_all[:q1, cj : cj + 1],
                                    acc_t[:q1],
                                    0.0,
                                    None,
                                    op0=OP.add,
                                )
                            continue
                        nc.vector.tensor_tensor(
                            d_t[:q1, h0:h1],
                            i_t[:q1, h0:h1],
                            p_t[:q1, h0:h1],
                            op=OP.subtract,
                        )
                        ab = abs_pool.tile([128, HIDDEN], f32)
                        nc.scalar.activation(
                            ab[:q1, :step],
                            d_t[:q1, h0:h1],
                            AF.Abs,
                            accum_out=sad_all[:q1, c0 : c0 + 1],
                        )
                elif cfg["halves"]:
                    # every unit split into column halves accumulating to
                    # paired cols (host adds): ACT runs 2 KB-granular and
                    # tracks DVE closely through the tail
                    for h, (h0, h1) in enumerate(((0, half), (half, HIDDEN))):
                        nc.vector.tensor_tensor(
                            d_t[:q1, h0:h1],
                            i_t[:q1, h0:h1],
                            p_t[:q1, h0:h1],
                            op=OP.subtract,
                        )
                        ab = abs_pool.tile([128, HIDDEN], f32)
                        nc.scalar.activation(
                            ab[:q1, :half],
                            d_t[:q1, h0:h1],
                            AF.Abs,
                            accum_out=sad_all[:q1, 2 * col + h : 2 * col + h + 1],
                        )
                elif split_c:
                    # split subtract+abs: ACT overlaps the second half, so
                    # first-unit compute starts / last-unit tail ends sooner
                    sadp = small_pool.tile([128, 2], f32, tag="sadp")
                    for h, (h0, h1) in enumerate(((0, half), (half, HIDDEN))):
                        nc.vector.tensor_tensor(
                            d_t[:q1, h0:h1],
                            i_t[:q1, h0:h1],
                            p_t[:q1, h0:h1],
                            op=OP.subtract,
                        )
                        ab = abs_pool.tile([128, HIDDEN], f32)
                        nc.scalar.activation(
                            ab[:q1, :half],
                            d_t[:q1, h0:h1],
                            AF.Abs,
                            accum_out=sadp[:q1, h : h + 1],
                        )
                    nc.vector.tensor_scalar(
                        sad_all[:q1, col : col + 1],
                        sadp[:q1, 0:1],
                        sadp[:q1, 1:2],
                        None,
                        op0=OP.add,
                    )
                else:
                    nc.vector.tensor_tensor(
                        d_t[:q1, :], i_t[:q1, :], p_t[:q1, :], op=OP.subtract
                    )
                    # |d| with full-width accumulate -> SAD scalar. Single
                    # 4096-elem f32 accumulation: rounding ~1.5e-2 absolute,
                    # below the min |SAD-thr| margin of ~3.4e-2 (verified:
                    # zero mask flips vs the f32 reference).
                    ab = abs_pool.tile([128, HIDDEN], f32)
                    nc.scalar.activation(
                        ab[:q1, :],
                        d_t[:q1, :],
                        AF.Abs,
                        accum_out=sad_all[:q1, col : col + 1],
                    )
                unit += 1

        # one tiny store of the SAD scalars per partition; early_store ships
        # the settled unit cols as soon as they are done so the final
        # DMAHW receipt wait covers only the last unit's piece cols
        store_eng = nc.sync if cfg["store"] == "sync" else nc.gpsimd
        if cfg["early_store"] and cfg["split_last_load"]:
            store_eng.dma_start(y_ap[:, :MASK_COLS], sad_all[:, :MASK_COLS])
            store_eng.dma_start(y_ap[:, MASK_COLS:], sad_all[:, MASK_COLS:])
        else:
            store_eng.dma_start(y_ap, sad_all)


@functools.cache
def _build_nc_cfg(cfg_key):
    import concourse.bacc as bacc
    import concourse.tile as tile
    from concourse import mybir

    cfg = dict(cfg_key)
    nc = bacc.Bacc(
        "TRN2",
        target_bir_lowering=False,
        debug=False,
        enable_asserts=False,
        num_devices=N_CORES,
    )
    x = nc.dram_tensor(
        "x", [TOK_PER_CORE, HIDDEN], mybir.dt.float32, kind="ExternalInput"
    ).ap()
    y = nc.dram_tensor(
        "y", [128, _layout(cfg)[1]], mybir.dt.float32, kind="ExternalOutput"
    ).ap()
    with tile.TileContext(nc) as tc:
        _kernel_body(tc, y, x, cfg)
    nc.compile()
    return nc


def _build_nc(cfg=None):
    cfg = cfg or _cfg()
    return _build_nc_cfg(tuple(sorted(cfg.items())))


def _in_maps(hs: np.ndarray):
    img = hs[0, IMG_START : IMG_START + IMG_LEN]
    maps = []
    for c in range(N_CORES):
        xc = img[TOK_PER_CORE * c : TOK_PER_CORE * (c + 1)]
        maps.append({"x": np.ascontiguousarray(xc)})
    return maps


def kernel(hidden_states: np.ndarray) -> np.ndarray:
    from concourse.bass_utils import run_bass_kernel_spmd

    hs = np.asarray(hidden_states, dtype=np.float32)
    assert hs.shape == (1, SEQ_LEN, HIDDEN), hs.shape
    nc = _build_nc()
    res = run_bass_kernel_spmd(nc, _in_maps(hs), list(range(N_CORES)))

    out = hs.copy()
    img = out[0, IMG_START : IMG_START + IMG_LEN].reshape(
        NUM_FRAMES, PATCHES, HIDDEN
    )
    src = hs[0, IMG_START : IMG_START + IMG_LEN].reshape(
        NUM_FRAMES, PATCHES, HIDDEN
    )
    cfg = _cfg()
    for c in range(N_CORES):
        # device returns raw f32 SAD scalars; f32 threshold compare here is
        # bit-identical to the reference's on-device decision
        mask_cols, _ = _layout(cfg)
        raw = res.results[c]["y"]  # [128, out_cols]
        if cfg["halves"]:
            # unit u = col pair (2u, 2u+1); f32 host add == device DVE add
            sad = raw[:, 0 : 2 * N_UNITS : 2] + raw[:, 1 : 2 * N_UNITS : 2]
        else:
            sad = raw[:, :N_UNITS].copy()
        if cfg["split_last_load"]:
            # last unit's SAD arrives in pieces (f32 left-to-right sum,
            # same chunked-accumulation error class as the device path);
            # TTR pieces hold (sum-max, sum-min) and contribute max - min
            ns = cfg["last_splits"]
            acc = np.zeros(raw.shape[0], dtype=np.float32)
            for j in range(ns):
                c0 = mask_cols + 2 * j
                if j >= ns - cfg["tail_ttr"]:
                    acc = acc + (raw[:, c0] - raw[:, c0 + 1])
                else:
                    acc = acc + raw[:, c0]
            sad[:, LAST_COL] = acc
        m = sad < np.float32(THRESHOLD)
        # guard band: decisions within GUARD of the threshold are re-derived
        # on host in f64, making the device accumulation order irrelevant
        near = np.abs(sad.astype(np.float64) - THRESHOLD) < GUARD
        ar = cfg["a_rows"]
        # host-side patches: A-window shed [ar:128) plus the %16 runt
        hp = list(range(ar, 128)) + list(range(RUNT_START, PATCHES))
        for iv in range(IVS_PER_CORE):
            gi = c * IVS_PER_CORE + iv
            fbase = gi * INTERVAL
            i_tok = src[fbase + I_POS]  # [PATCHES, HIDDEN]
            i64 = i_tok.astype(np.float64)
            # host patches: SAD on host (f64; margin >> f32 noise)
            runt = src[fbase : fbase + INTERVAL, hp, :]
            sad_r = np.abs(runt.astype(np.float64) - i64[hp][None]).sum(-1)
            for k in range(INTERVAL):
                if k == I_POS:
                    continue  # I-frame replaced by itself: no-op
                mk = np.empty(PATCHES, dtype=bool)
                mk[0:ar] = m[:ar, iv * 6 + k]
                mk[128:RUNT_START] = m[32:96, iv * 6 + 3 + k]
                mk[hp] = sad_r[k] < THRESHOLD
                for rows, poff in (
                    (np.nonzero(near[:ar, iv * 6 + k])[0], 0),
                    (32 + np.nonzero(near[32:96, iv * 6 + 3 + k])[0], 96),
                ):
                    for r in rows:
                        p = poff + r
                        d64 = src[fbase + k, p].astype(np.float64) - i64[p]
                        mk[p] = np.abs(d64).sum() < THRESHOLD
                img[fbase + k][mk] = i_tok[mk]
    return out

